# revision 7
# baseline (speedup 1.0000x reference)
"""Trainium2 Bass kernel for nn_Attention1 (channel attention, B=8,N=1024,C0=256,KV=512,H=4).

Sharding: pure data-parallel over batch B=8 across the 8 NeuronCores (one batch
element per core, no collectives).

Algorithm (per core, batch element b), algebraically refactored so the N=1024
dimension is contracted once up front:

    G    = emb_b^T @ emb_all_b                      [C0, KV]   (gram matrix)
    per head h:
      A_hT = G^T @ Wq_h^T                           [KV, C0]
      S_hT = Wk_h^T(^T) @ A_hT  (scores^T)          [KV, C0]
      alpha = 1/sqrt(var(S) + KV*eps)               (instance-norm; the mean
               cancels under softmax shift-invariance, so only var is needed;
               1/sqrt(KV) score scaling is folded into alpha)
      ET   = exp(alpha * S_hT)                      [KV, C0]
      Z_d  = sum_j ET[j, d]   (softmax denominators)
      U_h  = (ET^T @ Wv_h) / Z                      [C0, KV]
    Usum = sum_h U_h;  MT = Usum^T @ (Wout^T/H)     [KV, C0]
    o_b  = emb_all_b @ MT                           [N, C0]

This is exact (same math as the reference, ~3.6x fewer FLOPs) and needs zero
on-chip transposes: weights are pre-transposed on the host, and emb_all is
shipped both n-major (for G) and k-major (for the final projection).
"""

import sys

for _p in (
    "/root/.axon_site",
    "/root/.axon_site/_ro/trn_rl_repo",
    "/root/.axon_site/_ro/pypackages",
    "/opt/trn_rl_repo",
):
    if _p not in sys.path:
        sys.path.append(_p)

from contextlib import ExitStack

import numpy as np

import concourse.bass as bass
import concourse.tile as tile
from concourse import bacc, mybir
from concourse.bass_utils import run_bass_kernel_spmd

NCORES = 8
B, N, C0, KV, H = 8, 1024, 256, 512, 4
EPS = 1e-5
P = 128
NT, CT, KT = N // P, C0 // P, KV // P  # 8, 2, 4
CNT = C0 * KV  # elements per (b, h) score map

F32 = mybir.dt.float32


def _build_nc(mm_dtype: str = "float32r"):
    """Build + compile the single-core program (same program on all 8 cores)."""
    nc = bacc.Bacc(
        "TRN2",
        target_bir_lowering=False,
        debug=False,
        num_devices=NCORES,
    )

    MDT = getattr(mybir.dt, mm_dtype)

    emb_d = nc.dram_tensor("emb", [N, C0], MDT, kind="ExternalInput").ap()
    emb_all_d = nc.dram_tensor("emb_all", [N, KV], MDT, kind="ExternalInput").ap()
    emb_allT_d = nc.dram_tensor("emb_allT", [KV, N], MDT, kind="ExternalInput").ap()
    wqT_d = nc.dram_tensor("wqT", [H, C0, C0], MDT, kind="ExternalInput").ap()
    wkT_d = nc.dram_tensor("wkT", [H, KV, KV], MDT, kind="ExternalInput").ap()
    wv_d = nc.dram_tensor("wv", [H, KV, KV], MDT, kind="ExternalInput").ap()
    woutT_d = nc.dram_tensor("woutT", [C0, C0], MDT, kind="ExternalInput").ap()
    o_d = nc.dram_tensor("o", [N, C0], F32, kind="ExternalOutput").ap()

    def mv(ap):
        return ap

    Exp = mybir.ActivationFunctionType.Exp
    Sqrt = mybir.ActivationFunctionType.Sqrt

    with tile.TileContext(nc) as tc, ExitStack() as ctx:
        const = ctx.enter_context(tc.tile_pool(name="const", bufs=1))
        inp = ctx.enter_context(tc.tile_pool(name="inp", bufs=1))
        work = ctx.enter_context(tc.tile_pool(name="work", bufs=1))
        hwork = ctx.enter_context(tc.tile_pool(name="hwork", bufs=2))
        small = ctx.enter_context(tc.tile_pool(name="small", bufs=2))

        ones2_f = const.tile([P, 2], F32, tag="ones2_f")
        nc.vector.memset(ones2_f, 1.0)
        ones2 = const.tile([P, 2], MDT, tag="ones2")
        nc.vector.tensor_copy(ones2, ones2_f)
        ones_row = const.tile([1, P], F32, tag="ones_row")
        nc.vector.memset(ones_row, 1.0)
        epsb = const.tile([P, 1], F32, tag="epsb")
        nc.vector.memset(epsb, float(KV * EPS))

        # ---- resident input loads (order matters: feeds the pipeline) ----
        emb_sb = inp.tile([P, NT, C0], MDT, tag="emb")
        nc.sync.dma_start(out=emb_sb, in_=emb_d.rearrange("(t p) c -> p t c", p=P))
        emb_all_sb = inp.tile([P, NT, KV], MDT, tag="emb_all")
        nc.sync.dma_start(
            out=emb_all_sb, in_=emb_all_d.rearrange("(t p) c -> p t c", p=P)
        )
        wq_sb, wk_sb, wv_sb = [], [], []
        for h in range(H):
            t = inp.tile([P, CT, C0], MDT, tag=f"wq{h}")
            nc.sync.dma_start(out=t, in_=wqT_d[h].rearrange("(t p) c -> p t c", p=P))
            wq_sb.append(t)
            t = inp.tile([P, KT, KV], MDT, tag=f"wk{h}")
            nc.sync.dma_start(out=t, in_=wkT_d[h].rearrange("(t p) c -> p t c", p=P))
            wk_sb.append(t)
            t = inp.tile([P, KT, KV], MDT, tag=f"wv{h}")
            nc.sync.dma_start(out=t, in_=wv_d[h].rearrange("(t p) c -> p t c", p=P))
            wv_sb.append(t)
        woutT_sb = inp.tile([P, CT, C0], MDT, tag="woutT")
        nc.sync.dma_start(out=woutT_sb, in_=woutT_d.rearrange("(t p) c -> p t c", p=P))
        emb_allT_sb = inp.tile([P, KT, N], MDT, tag="emb_allT")
        nc.sync.dma_start(
            out=emb_allT_sb, in_=emb_allT_d.rearrange("(t p) c -> p t c", p=P)
        )

        # ---- phase 1: G = emb^T @ emb_all  -> [c(part,CT), kv] ----
        G_sb = work.tile([P, CT, KV], MDT, tag="G")
        with tc.tile_pool(name="psG", bufs=2, space="PSUM") as psG:
            for ct in range(CT):
                g_ps = psG.tile([P, KV], F32, tag="G")
                for nt in range(NT):
                    nc.tensor.matmul(
                        g_ps,
                        mv(emb_sb[:, nt, ct * P : (ct + 1) * P]),
                        mv(emb_all_sb[:, nt, :]),
                        start=(nt == 0),
                        stop=(nt == NT - 1),
                    )
                nc.scalar.copy(G_sb[:, ct, :], g_ps)

        # ---- phase 2: per-head attention in channel space ----
        Usum = work.tile([P, CT, KV], MDT, tag="Usum")
        with (
            tc.tile_pool(name="psA", bufs=2, space="PSUM") as psA,
            tc.tile_pool(name="psS", bufs=2, space="PSUM") as psS,
            tc.tile_pool(name="psU", bufs=2, space="PSUM") as psU,
            tc.tile_pool(name="psStat", bufs=2, space="PSUM") as psStat,
        ):
            for h in range(H):
                # A_hT[kv, d] = sum_c G[c, kv] * WqT[c, d]
                A_sb = hwork.tile([P, KT, C0], MDT, tag="A")
                for mt in range(KT):
                    a_ps = psA.tile([P, C0], F32, tag="A")
                    for kc in range(CT):
                        nc.tensor.matmul(
                            a_ps,
                            mv(G_sb[:, kc, mt * P : (mt + 1) * P]),
                            mv(wq_sb[h][:, kc, :]),
                            start=(kc == 0),
                            stop=(kc == CT - 1),
                        )
                    nc.scalar.copy(A_sb[:, mt, :], a_ps)

                # S_hT[j, d] = sum_kv WkT[kv, j] * A_hT[kv, d]; keep S and S^2
                SC = hwork.tile([P, KT, 2 * C0], MDT, tag="SC")
                for jm in range(KT):
                    s_ps = psS.tile([P, C0], F32, tag="S")
                    for kt in range(KT):
                        nc.tensor.matmul(
                            s_ps,
                            mv(wk_sb[h][:, kt, jm * P : (jm + 1) * P]),
                            mv(A_sb[:, kt, :]),
                            start=(kt == 0),
                            stop=(kt == KT - 1),
                        )
                    nc.vector.tensor_copy(SC[:, jm, 0:C0], s_ps)
                    nc.scalar.square(SC[:, jm, C0 : 2 * C0], s_ps)

                # grand sums of [S | S^2] over the whole map -> alpha
                cs_ps = psStat.tile([P, KV], F32, tag="stat")
                for jm in range(KT):
                    nc.tensor.matmul(
                        cs_ps[0:2, :],
                        ones2,
                        SC[:, jm, :],
                        start=(jm == 0),
                        stop=(jm == KT - 1),
                    )
                cs_sb = small.tile([1, KV], F32, tag="cs")
                nc.scalar.copy(cs_sb, cs_ps[0:1, :])
                bc_ps = psStat.tile([P, KV], F32, tag="stat")
                nc.tensor.matmul(bc_ps, ones_row, cs_sb, start=True, stop=True)
                s12 = small.tile([P, 2], F32, tag="s12")
                nc.vector.reduce_sum(
                    s12,
                    bc_ps.rearrange("p (a b) -> p a b", a=2),
                    axis=mybir.AxisListType.X,
                )
                m12 = small.tile([P, 2], F32, tag="m12")
                nc.vector.tensor_scalar_mul(m12, s12, 1.0 / CNT)
                var = small.tile([P, 1], F32, tag="var")
                nc.vector.tensor_mul(var, m12[:, 0:1], m12[:, 0:1])
                nc.vector.tensor_sub(var, m12[:, 1:2], var)
                sd = small.tile([P, 1], F32, tag="sd")
                nc.scalar.activation(sd, var, Sqrt, bias=epsb)
                alpha = small.tile([P, 1], F32, tag="alpha")
                nc.vector.reciprocal(alpha, sd)

                # ET = exp(alpha * S)
                ET = hwork.tile([P, KT, C0], MDT, tag="ET")
                for jm in range(KT):
                    nc.scalar.activation(
                        ET[:, jm, :], SC[:, jm, 0:C0], Exp, scale=alpha
                    )

                # Z[d] = sum_j ET[j, d]; zr = 1/Z
                zr = small.tile([P, CT], F32, tag="zr")
                for dm in range(CT):
                    z_ps = psStat.tile([P, KV], F32, tag="stat")
                    for jm in range(KT):
                        nc.tensor.matmul(
                            z_ps[:, 0:2],
                            ET[:, jm, dm * P : (dm + 1) * P],
                            ones2,
                            start=(jm == 0),
                            stop=(jm == KT - 1),
                        )
                    nc.vector.reciprocal(zr[:, dm : dm + 1], z_ps[:, 0:1])

                # U'[d, k] = sum_j ET[j, d] * Wv[j, k];  Usum += U' / Z
                for dm in range(CT):
                    u_ps = psU.tile([P, KV], F32, tag="U")
                    for jm in range(KT):
                        nc.tensor.matmul(
                            u_ps,
                            mv(ET[:, jm, dm * P : (dm + 1) * P]),
                            mv(wv_sb[h][:, jm, :]),
                            start=(jm == 0),
                            stop=(jm == KT - 1),
                        )
                    if h == 0:
                        nc.vector.tensor_scalar_mul(
                            Usum[:, dm, :], u_ps, zr[:, dm : dm + 1]
                        )
                    else:
                        ut = hwork.tile([P, KV], MDT, tag="ut")
                        nc.vector.tensor_scalar_mul(ut, u_ps, zr[:, dm : dm + 1])
                        nc.vector.tensor_add(Usum[:, dm, :], Usum[:, dm, :], ut)

        # ---- phase 3: MT[k, d'] = sum_d Usum[d, k] * WoutT[d, d'] ----
        MT_sb = work.tile([P, KT, C0], MDT, tag="MT")
        with tc.tile_pool(name="psMT", bufs=2, space="PSUM") as psMT:
            for km in range(KT):
                mt_ps = psMT.tile([P, C0], F32, tag="MT")
                for dt_ in range(CT):
                    nc.tensor.matmul(
                        mt_ps,
                        mv(Usum[:, dt_, km * P : (km + 1) * P]),
                        mv(woutT_sb[:, dt_, :]),
                        start=(dt_ == 0),
                        stop=(dt_ == CT - 1),
                    )
                nc.scalar.copy(MT_sb[:, km, :], mt_ps)

        # ---- phase 4: o[n, d'] = sum_k emb_allT[k, n] * MT[k, d'] ----
        with (
            tc.tile_pool(name="psO", bufs=3, space="PSUM") as psO,
            tc.tile_pool(name="osb", bufs=3) as osb,
        ):
            o_r = o_d.rearrange("(t p) c -> p t c", p=P)
            for nm in range(NT):
                o_ps = psO.tile([P, C0], F32, tag="o")
                for kt in range(KT):
                    nc.tensor.matmul(
                        o_ps,
                        mv(emb_allT_sb[:, kt, nm * P : (nm + 1) * P]),
                        mv(MT_sb[:, kt, :]),
                        start=(kt == 0),
                        stop=(kt == KT - 1),
                    )
                ot = osb.tile([P, C0], F32, tag="o")
                nc.scalar.copy(ot, o_ps)
                nc.sync.dma_start(out=o_r[:, nm, :], in_=ot)

    nc.compile()
    return nc


_NC_CACHE: dict = {}


def _get_nc(mm_dtype: str = "float32r"):
    if mm_dtype not in _NC_CACHE:
        _NC_CACHE[mm_dtype] = _build_nc(mm_dtype)
    return _NC_CACHE[mm_dtype]


def _make_in_maps(emb, emb_all, Wq, Wk, Wv, Wout):
    f = np.float32
    wqT = np.ascontiguousarray(np.asarray(Wq, f).transpose(0, 2, 1))
    wkT = np.ascontiguousarray(np.asarray(Wk, f).transpose(0, 2, 1))
    wv = np.ascontiguousarray(np.asarray(Wv, f))
    woutT = np.ascontiguousarray(np.asarray(Wout, f).T * (1.0 / H))
    in_maps = []
    for b in range(B):
        in_maps.append(
            dict(
                emb=np.ascontiguousarray(np.asarray(emb[b], f)),
                emb_all=np.ascontiguousarray(np.asarray(emb_all[b], f)),
                emb_allT=np.ascontiguousarray(np.asarray(emb_all[b], f).T),
                wqT=wqT,
                wkT=wkT,
                wv=wv,
                woutT=woutT,
            )
        )
    return in_maps


def run(inputs: dict, mm_dtype: str = "float32r", **spmd_kwargs):
    """Run on the 8 NeuronCores; returns (output [B,N,C0], BassKernelResults)."""
    nc = _get_nc(mm_dtype)
    in_maps = _make_in_maps(**inputs)
    res = run_bass_kernel_spmd(nc, in_maps, list(range(NCORES)), **spmd_kwargs)
    out = np.stack([res.results[c]["o"] for c in range(NCORES)], axis=0)
    return out, res


def kernel(emb, emb_all, Wq, Wk, Wv, Wout):
    out, _ = run(dict(emb=emb, emb_all=emb_all, Wq=Wq, Wk=Wk, Wv=Wv, Wout=Wout))
    return out


# revision 8
# speedup vs baseline: 1.0940x; 1.0940x over previous
"""Trainium2 Bass kernel for nn_Attention1 (channel attention, B=8,N=1024,C0=256,KV=512,H=4).

Sharding: pure data-parallel over batch B=8 across the 8 NeuronCores (one batch
element per core, no collectives).

Algorithm (per core, batch element b), algebraically refactored so the N=1024
dimension is contracted once up front:

    G    = emb_b^T @ emb_all_b                      [C0, KV]   (gram matrix)
    per head h:
      A_hT = G^T @ Wq_h^T                           [KV, C0]
      S_hT = (Wk_h^T)-chain @ A_hT  (scores^T)      [KV, C0]
      alpha = 1/sqrt(var(S) + KV*eps)               (instance-norm; the mean
               cancels under softmax shift-invariance, so only var is needed;
               the 1/sqrt(KV) score scaling is folded into alpha)
      ET   = exp(alpha * S_hT)                      [KV, C0]
      Z_d  = sum_j ET[j, d]   (softmax denominators)
      U_h  = (ET^T @ Wv_h) / Z                      [C0, KV]
    Usum = sum_h U_h;  MT = Usum^T @ (Wout^T/H)     [KV, C0]
    o_b  = emb_all_b @ MT                           [N, C0]

This is exact (same math as the reference, ~3.6x fewer FLOPs) and needs zero
on-chip transposes: weights are pre-transposed on the host, and emb_all is
shipped both n-major (for G) and k-major (for the final projection).

The head loop is software-pipelined in emission order (A0 S0 A1 St0 S1 A2 ZU0
St1 S2 A3 ZU1 St2 S3 ZU2 St3 ZU3) so the TensorE instruction stream never
stalls on a head's cross-engine stats/softmax chain.  Map-wide variance stats
are fused into the PSUM->SBUF copy/square via ScalarE accum_out (per-partition
row sums) followed by one tiny ones-matmul per head.
"""

import sys

for _p in (
    "/root/.axon_site",
    "/root/.axon_site/_ro/trn_rl_repo",
    "/root/.axon_site/_ro/pypackages",
    "/opt/trn_rl_repo",
):
    if _p not in sys.path:
        sys.path.append(_p)

from contextlib import ExitStack

import numpy as np

import concourse.bass as bass
import concourse.tile as tile
from concourse import bacc, mybir
from concourse.bass_utils import run_bass_kernel_spmd

NCORES = 8
B, N, C0, KV, H = 8, 1024, 256, 512, 4
EPS = 1e-5
P = 128
NT, CT, KT = N // P, C0 // P, KV // P  # 8, 2, 4
CNT = C0 * KV  # elements per (b, h) score map

F32 = mybir.dt.float32


def _build_nc(mm_dtype: str = "float16"):
    """Build + compile the single-core program (same program on all 8 cores)."""
    nc = bacc.Bacc(
        "TRN2",
        target_bir_lowering=False,
        debug=False,
        num_devices=NCORES,
    )

    MDT = getattr(mybir.dt, mm_dtype)

    emb_d = nc.dram_tensor("emb", [N, C0], MDT, kind="ExternalInput").ap()
    emb_all_d = nc.dram_tensor("emb_all", [N, KV], MDT, kind="ExternalInput").ap()
    emb_allT_d = nc.dram_tensor("emb_allT", [KV, N], MDT, kind="ExternalInput").ap()
    wqT_d = nc.dram_tensor("wqT", [H, C0, C0], MDT, kind="ExternalInput").ap()
    wkT_d = nc.dram_tensor("wkT", [H, KV, KV], MDT, kind="ExternalInput").ap()
    wv_d = nc.dram_tensor("wv", [H, KV, KV], MDT, kind="ExternalInput").ap()
    woutT_d = nc.dram_tensor("woutT", [C0, C0], MDT, kind="ExternalInput").ap()
    o_d = nc.dram_tensor("o", [N, C0], F32, kind="ExternalOutput").ap()

    Exp = mybir.ActivationFunctionType.Exp
    Sqrt = mybir.ActivationFunctionType.Sqrt
    Square = mybir.ActivationFunctionType.Square
    Copy = mybir.ActivationFunctionType.Copy

    with tile.TileContext(nc) as tc, ExitStack() as ctx:
        const = ctx.enter_context(tc.tile_pool(name="const", bufs=1))
        inp = ctx.enter_context(tc.tile_pool(name="inp", bufs=1))
        work = ctx.enter_context(tc.tile_pool(name="work", bufs=1))
        hwork = ctx.enter_context(tc.tile_pool(name="hwork", bufs=2))
        small = ctx.enter_context(tc.tile_pool(name="small", bufs=2))

        ones2_f = const.tile([P, 2], F32, tag="ones2_f")
        nc.vector.memset(ones2_f, 1.0)
        ones2 = const.tile([P, 2], MDT, tag="ones2")
        nc.vector.tensor_copy(ones2, ones2_f)
        ones_row_f = const.tile([1, P], F32, tag="ones_row_f")
        nc.vector.memset(ones_row_f, 1.0)
        ones_row = const.tile([1, P], MDT, tag="ones_row")
        nc.vector.tensor_copy(ones_row, ones_row_f)
        epsb = const.tile([P, 1], F32, tag="epsb")
        nc.vector.memset(epsb, float(KV * EPS))

        # ---- resident input loads (order matters: feeds the pipeline) ----
        emb_sb = inp.tile([P, NT, C0], MDT, tag="emb")
        nc.sync.dma_start(out=emb_sb, in_=emb_d.rearrange("(t p) c -> p t c", p=P))
        emb_all_sb = inp.tile([P, NT, KV], MDT, tag="emb_all")
        nc.sync.dma_start(
            out=emb_all_sb, in_=emb_all_d.rearrange("(t p) c -> p t c", p=P)
        )
        wq_sb, wk_sb, wv_sb = [], [], []
        for h in range(H):
            t = inp.tile([P, CT, C0], MDT, tag=f"wq{h}")
            nc.sync.dma_start(out=t, in_=wqT_d[h].rearrange("(t p) c -> p t c", p=P))
            wq_sb.append(t)
            t = inp.tile([P, KT, KV], MDT, tag=f"wk{h}")
            nc.sync.dma_start(out=t, in_=wkT_d[h].rearrange("(t p) c -> p t c", p=P))
            wk_sb.append(t)
            t = inp.tile([P, KT, KV], MDT, tag=f"wv{h}")
            nc.sync.dma_start(out=t, in_=wv_d[h].rearrange("(t p) c -> p t c", p=P))
            wv_sb.append(t)
        woutT_sb = inp.tile([P, CT, C0], MDT, tag="woutT")
        nc.sync.dma_start(out=woutT_sb, in_=woutT_d.rearrange("(t p) c -> p t c", p=P))
        emb_allT_sb = inp.tile([P, KT, N], MDT, tag="emb_allT")
        nc.sync.dma_start(
            out=emb_allT_sb, in_=emb_allT_d.rearrange("(t p) c -> p t c", p=P)
        )

        # ---- phase 1: G = emb^T @ emb_all  -> [c(part,CT), kv] ----
        G_sb = work.tile([P, CT, KV], MDT, tag="G")
        with tc.tile_pool(name="psG", bufs=2, space="PSUM") as psG:
            for ct in range(CT):
                g_ps = psG.tile([P, KV], F32, tag="G")
                for nt in range(NT):
                    nc.tensor.matmul(
                        g_ps,
                        emb_sb[:, nt, ct * P : (ct + 1) * P],
                        emb_all_sb[:, nt, :],
                        start=(nt == 0),
                        stop=(nt == NT - 1),
                    )
                nc.scalar.copy(G_sb[:, ct, :], g_ps)

        # ---- phase 2: per-head attention in channel space (sw-pipelined) ----
        Usum = work.tile([P, CT, KV], MDT, tag="Usum")
        A_t, SC_t, RS_t, ET_t, alpha_t, zr_t = {}, {}, {}, {}, {}, {}
        with (
            tc.tile_pool(name="psA", bufs=2, space="PSUM") as psA,
            tc.tile_pool(name="psS", bufs=2, space="PSUM") as psS,
            tc.tile_pool(name="psU", bufs=2, space="PSUM") as psU,
            tc.tile_pool(name="psSmall", bufs=2, space="PSUM") as psSmall,
        ):

            def emit_A(h):
                # A_hT[kv, d] = sum_c G[c, kv] * WqT[c, d]
                A_sb = hwork.tile([P, KT, C0], MDT, tag="A")
                A_t[h] = A_sb
                for mt in range(KT):
                    a_ps = psA.tile([P, C0], F32, tag="A")
                    for kc in range(CT):
                        nc.tensor.matmul(
                            a_ps,
                            G_sb[:, kc, mt * P : (mt + 1) * P],
                            wq_sb[h][:, kc, :],
                            start=(kc == 0),
                            stop=(kc == CT - 1),
                        )
                    nc.scalar.copy(A_sb[:, mt, :], a_ps)

            def emit_S(h):
                # S_hT[j, d] = sum_kv WkT[kv, j] * A_hT[kv, d]
                # fused stats: RS[:, jm] = rowsum(S), RS[:, KT+jm] = rowsum(S^2)
                SC = hwork.tile([P, KT, C0], MDT, tag="SC")
                RS = hwork.tile([P, 2 * KT], F32, tag="RS")
                SC_t[h], RS_t[h] = SC, RS
                A_sb = A_t[h]
                for jm in range(KT):
                    s_ps = psS.tile([P, C0], F32, tag="S")
                    for kt in range(KT):
                        nc.tensor.matmul(
                            s_ps,
                            wk_sb[h][:, kt, jm * P : (jm + 1) * P],
                            A_sb[:, kt, :],
                            start=(kt == 0),
                            stop=(kt == KT - 1),
                        )
                    nc.scalar.activation(
                        SC[:, jm, :], s_ps, Copy, accum_out=RS[:, jm : jm + 1]
                    )
                    sqscr = hwork.tile([P, C0], MDT, tag="sqscr")
                    nc.scalar.activation(
                        sqscr, s_ps, Square, accum_out=RS[:, KT + jm : KT + jm + 1]
                    )

            def emit_stats(h):
                # grand sums over partitions via one tiny ones-matmul,
                # then the scalar alpha chain on a [1,1] lane, broadcast back.
                cs_ps = psSmall.tile([P, 2 * KT], F32, tag="small")
                nc.tensor.matmul(
                    cs_ps[0:2, :], ones2_f, RS_t[h], start=True, stop=True
                )
                s12 = small.tile([1, 2], F32, tag="s12")
                nc.vector.reduce_sum(
                    s12,
                    cs_ps[0:1, :].rearrange("p (a b) -> p a b", a=2),
                    axis=mybir.AxisListType.X,
                )
                m12 = small.tile([1, 2], F32, tag="m12")
                nc.vector.tensor_scalar_mul(m12, s12, 1.0 / CNT)
                var = small.tile([1, 1], F32, tag="var")
                nc.vector.tensor_mul(var, m12[:, 0:1], m12[:, 0:1])
                nc.vector.tensor_sub(var, m12[:, 1:2], var)
                sd = small.tile([1, 1], F32, tag="sd")
                nc.scalar.activation(sd, var, Sqrt, bias=epsb[0:1, :])
                ar = small.tile([1, 1], F32, tag="ar")
                nc.vector.reciprocal(ar, sd)
                a2 = small.tile([1, 2], MDT, tag="a2")
                nc.vector.tensor_copy(a2[:, 0:1], ar)
                nc.vector.tensor_copy(a2[:, 1:2], ar)
                ab_ps = psSmall.tile([P, 2], F32, tag="small")
                nc.tensor.matmul(ab_ps, ones_row, a2, start=True, stop=True)
                alpha = small.tile([P, 1], F32, tag="alpha")
                nc.vector.tensor_copy(alpha, ab_ps[:, 0:1])
                alpha_t[h] = alpha
                # ET = exp(alpha * S)
                ET = hwork.tile([P, KT, C0], MDT, tag="ET")
                ET_t[h] = ET
                SC = SC_t[h]
                for jm in range(KT):
                    nc.scalar.activation(ET[:, jm, :], SC[:, jm, :], Exp, scale=alpha)

            def emit_ZU(h):
                ET = ET_t[h]
                # Z[d] = sum_j ET[j, d]; zr = 1/Z
                zr = small.tile([P, CT], F32, tag="zr")
                zr_t[h] = zr
                for dm in range(CT):
                    z_ps = psSmall.tile([P, 2], F32, tag="small")
                    for jm in range(KT):
                        nc.tensor.matmul(
                            z_ps,
                            ET[:, jm, dm * P : (dm + 1) * P],
                            ones2,
                            start=(jm == 0),
                            stop=(jm == KT - 1),
                        )
                    nc.vector.reciprocal(zr[:, dm : dm + 1], z_ps[:, 0:1])
                # U'[d, k] = sum_j ET[j, d] * Wv[j, k];  Usum += U' / Z
                for dm in range(CT):
                    u_ps = psU.tile([P, KV], F32, tag="U")
                    for jm in range(KT):
                        nc.tensor.matmul(
                            u_ps,
                            ET[:, jm, dm * P : (dm + 1) * P],
                            wv_sb[h][:, jm, :],
                            start=(jm == 0),
                            stop=(jm == KT - 1),
                        )
                    if h == 0:
                        nc.vector.tensor_scalar_mul(
                            Usum[:, dm, :], u_ps, zr[:, dm : dm + 1]
                        )
                    else:
                        ut = hwork.tile([P, KV], MDT, tag="ut")
                        nc.vector.tensor_scalar_mul(ut, u_ps, zr[:, dm : dm + 1])
                        nc.vector.tensor_add(Usum[:, dm, :], Usum[:, dm, :], ut)

            # skewed emission: PE stream never waits on a head's stats chain
            emit_A(0)
            emit_S(0)
            emit_A(1)
            emit_stats(0)
            emit_S(1)
            emit_A(2)
            emit_ZU(0)
            emit_stats(1)
            emit_S(2)
            emit_A(3)
            emit_ZU(1)
            emit_stats(2)
            emit_S(3)
            emit_ZU(2)
            emit_stats(3)
            emit_ZU(3)

        # ---- phase 3: MT[k, d'] = sum_d Usum[d, k] * WoutT[d, d'] ----
        MT_sb = work.tile([P, KT, C0], MDT, tag="MT")
        with tc.tile_pool(name="psMT", bufs=2, space="PSUM") as psMT:
            for km in range(KT):
                mt_ps = psMT.tile([P, C0], F32, tag="MT")
                for dt_ in range(CT):
                    nc.tensor.matmul(
                        mt_ps,
                        Usum[:, dt_, km * P : (km + 1) * P],
                        woutT_sb[:, dt_, :],
                        start=(dt_ == 0),
                        stop=(dt_ == CT - 1),
                    )
                nc.scalar.copy(MT_sb[:, km, :], mt_ps)

        # ---- phase 4: o[n, d'] = sum_k emb_allT[k, n] * MT[k, d'] ----
        with (
            tc.tile_pool(name="psO", bufs=3, space="PSUM") as psO,
            tc.tile_pool(name="osb", bufs=3) as osb,
        ):
            o_r = o_d.rearrange("(t p) c -> p t c", p=P)
            for nm in range(NT):
                o_ps = psO.tile([P, C0], F32, tag="o")
                for kt in range(KT):
                    nc.tensor.matmul(
                        o_ps,
                        emb_allT_sb[:, kt, nm * P : (nm + 1) * P],
                        MT_sb[:, kt, :],
                        start=(kt == 0),
                        stop=(kt == KT - 1),
                    )
                ot = osb.tile([P, C0], F32, tag="o")
                nc.scalar.copy(ot, o_ps)
                nc.sync.dma_start(out=o_r[:, nm, :], in_=ot)

    nc.compile()
    return nc


_NC_CACHE: dict = {}


def _get_nc(mm_dtype: str = "float16"):
    if mm_dtype not in _NC_CACHE:
        _NC_CACHE[mm_dtype] = _build_nc(mm_dtype)
    return _NC_CACHE[mm_dtype]


def _make_in_maps(emb, emb_all, Wq, Wk, Wv, Wout, np_dt=np.float16):
    f = np.float32
    wqT = np.ascontiguousarray(np.asarray(Wq, f).transpose(0, 2, 1)).astype(np_dt)
    wkT = np.ascontiguousarray(np.asarray(Wk, f).transpose(0, 2, 1)).astype(np_dt)
    wv = np.ascontiguousarray(np.asarray(Wv, f)).astype(np_dt)
    woutT = np.ascontiguousarray(np.asarray(Wout, f).T * (1.0 / H)).astype(np_dt)
    in_maps = []
    for b in range(B):
        in_maps.append(
            dict(
                emb=np.asarray(emb[b], f).astype(np_dt),
                emb_all=np.asarray(emb_all[b], f).astype(np_dt),
                emb_allT=np.ascontiguousarray(np.asarray(emb_all[b], f).T).astype(
                    np_dt
                ),
                wqT=wqT,
                wkT=wkT,
                wv=wv,
                woutT=woutT,
            )
        )
    return in_maps


def run(inputs: dict, mm_dtype: str = "float16", **spmd_kwargs):
    """Run on the 8 NeuronCores; returns (output [B,N,C0], BassKernelResults)."""
    nc = _get_nc(mm_dtype)
    np_dt = mybir.dt.np(getattr(mybir.dt, mm_dtype))
    in_maps = _make_in_maps(**inputs, np_dt=np_dt)
    res = run_bass_kernel_spmd(nc, in_maps, list(range(NCORES)), **spmd_kwargs)
    out = np.stack([res.results[c]["o"] for c in range(NCORES)], axis=0)
    return out, res


def kernel(emb, emb_all, Wq, Wk, Wv, Wout):
    out, _ = run(dict(emb=emb, emb_all=emb_all, Wq=Wq, Wk=Wk, Wv=Wv, Wout=Wout))
    return out


# revision 10
# speedup vs baseline: 1.2246x; 1.1194x over previous
"""Trainium2 Bass kernel for nn_Attention1 (channel attention, B=8,N=1024,C0=256,KV=512,H=4).

Sharding: pure data-parallel over batch B=8 across the 8 NeuronCores (one batch
element per core, no collectives).

Algorithm (per core, batch element b), algebraically refactored so the N=1024
dimension is contracted once up front:

    G    = emb_b^T @ emb_all_b                      [C0, KV]   (gram matrix)
    per head h:
      A_hT = G^T @ Wq_h^T                           [KV, C0]
      S_hT = (Wk_h^T)-chain @ A_hT  (scores^T)      [KV, C0]
      alpha = 1/sqrt(var(S) + KV*eps)               (instance-norm; the mean
               cancels under softmax shift-invariance, so only var is needed;
               the 1/sqrt(KV) score scaling is folded into alpha)
      ET   = exp(alpha * S_hT)                      [KV, C0]
      Z_d  = sum_j ET[j, d]   (softmax denominators)
      U_h  = (ET^T @ Wv_h) / Z                      [C0, KV]
    Usum = sum_h U_h;  MT = Usum^T @ (Wout^T/H)     [KV, C0]
    o_b  = emb_all_b @ MT                           [N, C0]

This is exact (same math as the reference, ~3.6x fewer FLOPs) and needs zero
on-chip transposes: weights are pre-transposed on the host, and emb_all is
shipped both n-major (for G) and k-major (for the final projection).

The head loop is software-pipelined in emission order (A0 S0 A1 St0 S1 A2 ZU0
St1 S2 A3 ZU1 St2 S3 ZU2 St3 ZU3) so the TensorE instruction stream never
stalls on a head's cross-engine stats/softmax chain.  Map-wide variance stats
are fused into the PSUM->SBUF copy/square via ScalarE accum_out (per-partition
row sums) followed by one tiny ones-matmul per head.
"""

import sys

for _p in (
    "/root/.axon_site",
    "/root/.axon_site/_ro/trn_rl_repo",
    "/root/.axon_site/_ro/pypackages",
    "/opt/trn_rl_repo",
):
    if _p not in sys.path:
        sys.path.append(_p)

from contextlib import ExitStack

import numpy as np

import concourse.bass as bass
import concourse.tile as tile
from concourse import bacc, mybir
from concourse.bass_utils import run_bass_kernel_spmd

NCORES = 8
B, N, C0, KV, H = 8, 1024, 256, 512, 4
EPS = 1e-5
P = 128
NT, CT, KT = N // P, C0 // P, KV // P  # 8, 2, 4
CNT = C0 * KV  # elements per (b, h) score map

F32 = mybir.dt.float32


def _build_nc(mm_dtype: str = "float16"):
    """Build + compile the single-core program (same program on all 8 cores)."""
    nc = bacc.Bacc(
        "TRN2",
        target_bir_lowering=False,
        debug=False,
        num_devices=NCORES,
    )

    MDT = getattr(mybir.dt, mm_dtype)

    emb_d = nc.dram_tensor("emb", [N, C0], MDT, kind="ExternalInput").ap()
    emb_all_d = nc.dram_tensor("emb_all", [N, KV], MDT, kind="ExternalInput").ap()
    emb_allT_d = nc.dram_tensor("emb_allT", [KV, N], MDT, kind="ExternalInput").ap()
    wqT_d = nc.dram_tensor("wqT", [H, C0, C0], MDT, kind="ExternalInput").ap()
    wkT_d = nc.dram_tensor("wkT", [H, KV, KV], MDT, kind="ExternalInput").ap()
    wv_d = nc.dram_tensor("wv", [H, KV, KV], MDT, kind="ExternalInput").ap()
    woutT_d = nc.dram_tensor("woutT", [C0, C0], MDT, kind="ExternalInput").ap()
    o_d = nc.dram_tensor("o", [N, C0], F32, kind="ExternalOutput").ap()

    Exp = mybir.ActivationFunctionType.Exp
    Sqrt = mybir.ActivationFunctionType.Sqrt
    Square = mybir.ActivationFunctionType.Square
    Copy = mybir.ActivationFunctionType.Copy

    with tile.TileContext(nc) as tc, ExitStack() as ctx:
        const = ctx.enter_context(tc.tile_pool(name="const", bufs=1))
        inp = ctx.enter_context(tc.tile_pool(name="inp", bufs=1))
        work = ctx.enter_context(tc.tile_pool(name="work", bufs=1))
        hwork = ctx.enter_context(tc.tile_pool(name="hwork", bufs=2))
        small = ctx.enter_context(tc.tile_pool(name="small", bufs=2))

        ones2_f = const.tile([P, 2], F32, tag="ones2_f")
        nc.vector.memset(ones2_f, 1.0)
        ones2 = const.tile([P, 2], MDT, tag="ones2")
        nc.vector.tensor_copy(ones2, ones2_f)
        ones_row_f = const.tile([1, P], F32, tag="ones_row_f")
        nc.vector.memset(ones_row_f, 1.0)
        ones_row = const.tile([1, P], MDT, tag="ones_row")
        nc.vector.tensor_copy(ones_row, ones_row_f)
        epsb = const.tile([P, 1], F32, tag="epsb")
        nc.vector.memset(epsb, float(KV * EPS))

        # ---- resident input loads (order matters: feeds the pipeline) ----
        emb_sb = inp.tile([P, NT, C0], MDT, tag="emb")
        nc.sync.dma_start(out=emb_sb, in_=emb_d.rearrange("(p t) c -> p t c", p=P))
        emb_all_sb = inp.tile([P, NT, KV], MDT, tag="emb_all")
        nc.sync.dma_start(
            out=emb_all_sb, in_=emb_all_d.rearrange("(p t) c -> p t c", p=P)
        )
        wq_sb, wk_sb, wv_sb = [], [], []
        for h in range(H):
            t = inp.tile([P, CT, C0], MDT, tag=f"wq{h}")
            nc.sync.dma_start(out=t, in_=wqT_d[h].rearrange("(t p) c -> p t c", p=P))
            wq_sb.append(t)
            t = inp.tile([P, KT, KV], MDT, tag=f"wk{h}")
            nc.sync.dma_start(out=t, in_=wkT_d[h].rearrange("(t p) c -> p t c", p=P))
            wk_sb.append(t)
            t = inp.tile([P, KT, KV], MDT, tag=f"wv{h}")
            nc.sync.dma_start(out=t, in_=wv_d[h].rearrange("(t p) c -> p t c", p=P))
            wv_sb.append(t)
        woutT_sb = inp.tile([P, CT, C0], MDT, tag="woutT")
        nc.sync.dma_start(out=woutT_sb, in_=woutT_d.rearrange("(t p) c -> p t c", p=P))
        emb_allT_sb = inp.tile([P, KT, N], MDT, tag="emb_allT")
        nc.sync.dma_start(
            out=emb_allT_sb, in_=emb_allT_d.rearrange("(t p) c -> p t c", p=P)
        )

        # ---- phase 1: G = emb^T @ emb_all  -> [c(part,CT), kv] ----
        G_sb = work.tile([P, CT, KV], MDT, tag="G")
        with tc.tile_pool(name="psG", bufs=2, space="PSUM") as psG:
            for ct in range(CT):
                g_ps = psG.tile([P, KV], F32, tag="G")
                for nt in range(NT):
                    nc.tensor.matmul(
                        g_ps,
                        emb_sb[:, nt, ct * P : (ct + 1) * P],
                        emb_all_sb[:, nt, :],
                        start=(nt == 0),
                        stop=(nt == NT - 1),
                    )
                nc.vector.tensor_copy(G_sb[:, ct, :], g_ps)

        # ---- phase 2: per-head attention in channel space (sw-pipelined) ----
        Usum = work.tile([P, CT, KV], MDT, tag="Usum")
        A_t, SC_t, RS_t, ET_t, alpha_t, zr_t = {}, {}, {}, {}, {}, {}
        with (
            tc.tile_pool(name="psA", bufs=2, space="PSUM") as psA,
            tc.tile_pool(name="psS", bufs=2, space="PSUM") as psS,
            tc.tile_pool(name="psU", bufs=2, space="PSUM") as psU,
            tc.tile_pool(name="psSmall", bufs=2, space="PSUM") as psSmall,
        ):

            def emit_A(h):
                # A_hT[kv, d] = sum_c G[c, kv] * WqT[c, d]
                A_sb = hwork.tile([P, KT, C0], MDT, tag="A")
                A_t[h] = A_sb
                for mt in range(KT):
                    a_ps = psA.tile([P, C0], F32, tag="A")
                    for kc in range(CT):
                        nc.tensor.matmul(
                            a_ps,
                            G_sb[:, kc, mt * P : (mt + 1) * P],
                            wq_sb[h][:, kc, :],
                            start=(kc == 0),
                            stop=(kc == CT - 1),
                        )
                    nc.vector.tensor_copy(A_sb[:, mt, :], a_ps)

            def emit_S(h):
                # S_hT[j, d] = sum_kv WkT[kv, j] * A_hT[kv, d]
                # fused stats: RS[:, jm] = rowsum(S), RS[:, KT+jm] = rowsum(S^2)
                SC = hwork.tile([P, KT, C0], MDT, tag="SC")
                RS = hwork.tile([P, 2 * KT], F32, tag="RS")
                SC_t[h], RS_t[h] = SC, RS
                A_sb = A_t[h]
                for jm in range(KT):
                    s_ps = psS.tile([P, C0], F32, tag="S")
                    for kt in range(KT):
                        nc.tensor.matmul(
                            s_ps,
                            wk_sb[h][:, kt, jm * P : (jm + 1) * P],
                            A_sb[:, kt, :],
                            start=(kt == 0),
                            stop=(kt == KT - 1),
                        )
                    nc.scalar.activation(
                        SC[:, jm, :], s_ps, Copy, accum_out=RS[:, jm : jm + 1]
                    )
                    sqscr = hwork.tile([P, C0], MDT, tag="sqscr")
                    nc.scalar.activation(
                        sqscr, s_ps, Square, accum_out=RS[:, KT + jm : KT + jm + 1]
                    )

            a2_t = {}

            def emit_stats1(h):
                # grand sums over partitions via one tiny ones-matmul,
                # then the scalar alpha chain on a [1,1] lane.
                cs_ps = psSmall.tile([P, 2 * KT], F32, tag="small")
                nc.tensor.matmul(
                    cs_ps[0:2, :], ones2_f, RS_t[h], start=True, stop=True
                )
                s12 = small.tile([1, 2], F32, tag="s12")
                nc.vector.reduce_sum(
                    s12,
                    cs_ps[0:1, :].rearrange("p (a b) -> p a b", a=2),
                    axis=mybir.AxisListType.X,
                )
                m12 = small.tile([1, 2], F32, tag="m12")
                nc.vector.tensor_scalar_mul(m12, s12, 1.0 / CNT)
                var = small.tile([1, 1], F32, tag="var")
                nc.vector.tensor_mul(var, m12[:, 0:1], m12[:, 0:1])
                nc.vector.tensor_sub(var, m12[:, 1:2], var)
                sd = small.tile([1, 1], F32, tag="sd")
                nc.scalar.activation(sd, var, Sqrt, bias=epsb[0:1, :])
                ar = small.tile([1, 1], F32, tag="ar")
                nc.vector.reciprocal(ar, sd)
                a2 = small.tile([1, 2], MDT, tag="a2")
                nc.vector.tensor_copy(a2[:, 0:1], ar)
                nc.vector.tensor_copy(a2[:, 1:2], ar)
                a2_t[h] = a2

            def emit_stats2(h):
                # broadcast alpha to all partitions, then ET = exp(alpha * S)
                ab_ps = psSmall.tile([P, 2], F32, tag="small")
                nc.tensor.matmul(ab_ps, ones_row, a2_t[h], start=True, stop=True)
                alpha = small.tile([P, 1], F32, tag="alpha")
                nc.vector.tensor_copy(alpha, ab_ps[:, 0:1])
                alpha_t[h] = alpha
                ET = hwork.tile([P, KT, C0], MDT, tag="ET")
                ET_t[h] = ET
                SC = SC_t[h]
                for jm in range(KT):
                    nc.scalar.activation(ET[:, jm, :], SC[:, jm, :], Exp, scale=alpha)

            def emit_ZU(h):
                ET = ET_t[h]
                # Z[d] = sum_j ET[j, d]; zr = 1/Z
                zr = small.tile([P, CT], F32, tag="zr")
                zr_t[h] = zr
                for dm in range(CT):
                    z_ps = psSmall.tile([P, 2], F32, tag="small")
                    for jm in range(KT):
                        nc.tensor.matmul(
                            z_ps,
                            ET[:, jm, dm * P : (dm + 1) * P],
                            ones2,
                            start=(jm == 0),
                            stop=(jm == KT - 1),
                        )
                    nc.vector.reciprocal(zr[:, dm : dm + 1], z_ps[:, 0:1])
                # U'[d, k] = sum_j ET[j, d] * Wv[j, k];  Usum += U' / Z
                for dm in range(CT):
                    u_ps = psU.tile([P, KV], F32, tag="U")
                    for jm in range(KT):
                        nc.tensor.matmul(
                            u_ps,
                            ET[:, jm, dm * P : (dm + 1) * P],
                            wv_sb[h][:, jm, :],
                            start=(jm == 0),
                            stop=(jm == KT - 1),
                        )
                    if h == 0:
                        nc.vector.tensor_scalar_mul(
                            Usum[:, dm, :], u_ps, zr[:, dm : dm + 1]
                        )
                    else:
                        ut = hwork.tile([P, KV], MDT, tag="ut")
                        nc.vector.tensor_scalar_mul(ut, u_ps, zr[:, dm : dm + 1])
                        nc.vector.tensor_add(Usum[:, dm, :], Usum[:, dm, :], ut)

            # skewed emission: PE stream never waits on a head's stats chain
            emit_A(0)
            emit_S(0)
            emit_A(1)
            emit_stats1(0)
            emit_S(1)
            emit_stats2(0)
            emit_A(2)
            emit_stats1(1)
            emit_ZU(0)
            emit_S(2)
            emit_stats2(1)
            emit_A(3)
            emit_stats1(2)
            emit_ZU(1)
            emit_S(3)
            emit_stats2(2)
            emit_ZU(2)
            emit_stats1(3)
            emit_stats2(3)
            emit_ZU(3)

        # ---- phase 3: MT[k, d'] = sum_d Usum[d, k] * WoutT[d, d'] ----
        MT_sb = work.tile([P, KT, C0], MDT, tag="MT")
        with tc.tile_pool(name="psMT", bufs=2, space="PSUM") as psMT:
            for km in range(KT):
                mt_ps = psMT.tile([P, C0], F32, tag="MT")
                for dt_ in range(CT):
                    nc.tensor.matmul(
                        mt_ps,
                        Usum[:, dt_, km * P : (km + 1) * P],
                        woutT_sb[:, dt_, :],
                        start=(dt_ == 0),
                        stop=(dt_ == CT - 1),
                    )
                nc.scalar.copy(MT_sb[:, km, :], mt_ps)

        # ---- phase 4: o[n, d'] = sum_k emb_allT[k, n] * MT[k, d'] ----
        with (
            tc.tile_pool(name="psO", bufs=3, space="PSUM") as psO,
            tc.tile_pool(name="osb", bufs=3) as osb,
        ):
            o_r = o_d.rearrange("(t p) c -> p t c", p=P)
            for nm in range(NT):
                o_ps = psO.tile([P, C0], F32, tag="o")
                for kt in range(KT):
                    nc.tensor.matmul(
                        o_ps,
                        emb_allT_sb[:, kt, nm * P : (nm + 1) * P],
                        MT_sb[:, kt, :],
                        start=(kt == 0),
                        stop=(kt == KT - 1),
                    )
                ot = osb.tile([P, C0], F32, tag="o")
                if nm % 2 == 0:
                    nc.vector.tensor_copy(ot, o_ps)
                else:
                    nc.scalar.copy(ot, o_ps)
                nc.sync.dma_start(out=o_r[:, nm, :], in_=ot)

    nc.compile()
    return nc


_NC_CACHE: dict = {}


def _get_nc(mm_dtype: str = "float16"):
    if mm_dtype not in _NC_CACHE:
        _NC_CACHE[mm_dtype] = _build_nc(mm_dtype)
    return _NC_CACHE[mm_dtype]


def _make_in_maps(emb, emb_all, Wq, Wk, Wv, Wout, np_dt=np.float16):
    f = np.float32
    wqT = np.ascontiguousarray(np.asarray(Wq, f).transpose(0, 2, 1)).astype(np_dt)
    wkT = np.ascontiguousarray(np.asarray(Wk, f).transpose(0, 2, 1)).astype(np_dt)
    wv = np.ascontiguousarray(np.asarray(Wv, f)).astype(np_dt)
    woutT = np.ascontiguousarray(np.asarray(Wout, f).T * (1.0 / H)).astype(np_dt)
    in_maps = []
    for b in range(B):
        in_maps.append(
            dict(
                emb=np.asarray(emb[b], f).astype(np_dt),
                emb_all=np.asarray(emb_all[b], f).astype(np_dt),
                emb_allT=np.ascontiguousarray(np.asarray(emb_all[b], f).T).astype(
                    np_dt
                ),
                wqT=wqT,
                wkT=wkT,
                wv=wv,
                woutT=woutT,
            )
        )
    return in_maps


def run(inputs: dict, mm_dtype: str = "float16", **spmd_kwargs):
    """Run on the 8 NeuronCores; returns (output [B,N,C0], BassKernelResults)."""
    nc = _get_nc(mm_dtype)
    np_dt = mybir.dt.np(getattr(mybir.dt, mm_dtype))
    in_maps = _make_in_maps(**inputs, np_dt=np_dt)
    res = run_bass_kernel_spmd(nc, in_maps, list(range(NCORES)), **spmd_kwargs)
    out = np.stack([res.results[c]["o"] for c in range(NCORES)], axis=0)
    return out, res


def kernel(emb, emb_all, Wq, Wk, Wv, Wout):
    out, _ = run(dict(emb=emb, emb_all=emb_all, Wq=Wq, Wk=Wk, Wv=Wv, Wout=Wout))
    return out


# revision 11
# speedup vs baseline: 1.4465x; 1.1812x over previous
"""Trainium2 Bass kernel for nn_Attention1 (channel attention, B=8,N=1024,C0=256,KV=512,H=4).

Sharding: pure data-parallel over batch B=8 across the 8 NeuronCores (one batch
element per core, no collectives).

Algorithm (per core, batch element b), algebraically refactored so the N=1024
dimension is contracted once up front:

    G    = emb_b^T @ emb_all_b                      [C0, KV]   (gram matrix)
    per head h:
      A_hT = G^T @ Wq_h^T                           [KV, C0]
      S_hT = (Wk_h^T)-chain @ A_hT  (scores^T)      [KV, C0]
      alpha = 1/sqrt(var(S) + KV*eps)               (instance-norm; the mean
               cancels under softmax shift-invariance, so only var is needed;
               the 1/sqrt(KV) score scaling is folded into alpha)
      ET   = exp(alpha * S_hT)                      [KV, C0]
      Z_d  = sum_j ET[j, d]   (softmax denominators)
      U_h  = (ET^T @ Wv_h) / Z                      [C0, KV]
    Usum = sum_h U_h;  MT = Usum^T @ (Wout^T/H)     [KV, C0]
    o_b  = emb_all_b @ MT                           [N, C0]

This is exact (same math as the reference, ~3.6x fewer FLOPs) and needs zero
on-chip transposes: weights are pre-transposed on the host, and emb_all is
shipped both n-major (for G) and k-major (for the final projection).

The head loop is software-pipelined in emission order (A0 S0 A1 St0 S1 A2 ZU0
St1 S2 A3 ZU1 St2 S3 ZU2 St3 ZU3) so the TensorE instruction stream never
stalls on a head's cross-engine stats/softmax chain.  Map-wide variance stats
are fused into the PSUM->SBUF copy/square via ScalarE accum_out (per-partition
row sums) followed by one tiny ones-matmul per head.
"""

import sys

for _p in (
    "/root/.axon_site",
    "/root/.axon_site/_ro/trn_rl_repo",
    "/root/.axon_site/_ro/pypackages",
    "/opt/trn_rl_repo",
):
    if _p not in sys.path:
        sys.path.append(_p)

from contextlib import ExitStack

import numpy as np

import concourse.bass as bass
import concourse.tile as tile
from concourse import bacc, mybir
from concourse.bass_utils import run_bass_kernel_spmd

NCORES = 8
B, N, C0, KV, H = 8, 1024, 256, 512, 4
EPS = 1e-5
P = 128
NT, CT, KT = N // P, C0 // P, KV // P  # 8, 2, 4
CNT = C0 * KV  # elements per (b, h) score map

F32 = mybir.dt.float32


def _build_nc(mm_dtype: str = "float16"):
    """Build + compile the single-core program (same program on all 8 cores)."""
    nc = bacc.Bacc(
        "TRN2",
        target_bir_lowering=False,
        debug=False,
        num_devices=NCORES,
    )

    MDT = getattr(mybir.dt, mm_dtype)

    emb_d = nc.dram_tensor("emb", [N, C0], MDT, kind="ExternalInput").ap()
    emb_all_d = nc.dram_tensor("emb_all", [N, KV], MDT, kind="ExternalInput").ap()
    emb_allT_d = nc.dram_tensor("emb_allT", [KV, N], MDT, kind="ExternalInput").ap()
    wqT_d = nc.dram_tensor("wqT", [H, C0, C0], MDT, kind="ExternalInput").ap()
    wkT_d = nc.dram_tensor("wkT", [H, KV, KV], MDT, kind="ExternalInput").ap()
    wv_d = nc.dram_tensor("wv", [H, KV, KV], MDT, kind="ExternalInput").ap()
    woutT_d = nc.dram_tensor("woutT", [C0, C0], MDT, kind="ExternalInput").ap()
    o_d = nc.dram_tensor("o", [N, C0], F32, kind="ExternalOutput").ap()

    Exp = mybir.ActivationFunctionType.Exp
    Sqrt = mybir.ActivationFunctionType.Sqrt
    Square = mybir.ActivationFunctionType.Square
    Copy = mybir.ActivationFunctionType.Copy

    with tile.TileContext(nc) as tc, ExitStack() as ctx:
        const = ctx.enter_context(tc.tile_pool(name="const", bufs=1))
        inp = ctx.enter_context(tc.tile_pool(name="inp", bufs=1))
        work = ctx.enter_context(tc.tile_pool(name="work", bufs=1))
        hwork = ctx.enter_context(tc.tile_pool(name="hwork", bufs=2))
        small = ctx.enter_context(tc.tile_pool(name="small", bufs=2))

        ones2_f = const.tile([P, 2], F32, tag="ones2_f")
        nc.vector.memset(ones2_f, 1.0)
        ones2 = const.tile([P, 2], MDT, tag="ones2")
        nc.vector.tensor_copy(ones2, ones2_f)
        ones_row_f = const.tile([1, P], F32, tag="ones_row_f")
        nc.vector.memset(ones_row_f, 1.0)
        ones_row = const.tile([1, P], MDT, tag="ones_row")
        nc.vector.tensor_copy(ones_row, ones_row_f)
        epsb = const.tile([P, 1], F32, tag="epsb")
        nc.vector.memset(epsb, float(KV * EPS))

        # ---- resident input loads (order matters: feeds the pipeline) ----
        # per-tile chunks so the G matmuls can start as soon as chunk 0 lands
        emb_sb = inp.tile([P, NT, C0], MDT, tag="emb")
        emb_r = emb_d.rearrange("(t p) c -> p t c", p=P)
        emb_all_sb = inp.tile([P, NT, KV], MDT, tag="emb_all")
        emb_all_r = emb_all_d.rearrange("(t p) c -> p t c", p=P)
        for nt in range(NT):
            nc.sync.dma_start(out=emb_sb[:, nt, :], in_=emb_r[:, nt, :])
            nc.sync.dma_start(out=emb_all_sb[:, nt, :], in_=emb_all_r[:, nt, :])
        wq_sb, wk_sb, wv_sb = [], [], []
        for h in range(H):
            t = inp.tile([P, CT, C0], MDT, tag=f"wq{h}")
            nc.sync.dma_start(out=t, in_=wqT_d[h].rearrange("(t p) c -> p t c", p=P))
            wq_sb.append(t)
            t = inp.tile([P, KT, KV], MDT, tag=f"wk{h}")
            nc.sync.dma_start(out=t, in_=wkT_d[h].rearrange("(t p) c -> p t c", p=P))
            wk_sb.append(t)
            t = inp.tile([P, KT, KV], MDT, tag=f"wv{h}")
            nc.sync.dma_start(out=t, in_=wv_d[h].rearrange("(t p) c -> p t c", p=P))
            wv_sb.append(t)
        woutT_sb = inp.tile([P, CT, C0], MDT, tag="woutT")
        nc.sync.dma_start(out=woutT_sb, in_=woutT_d.rearrange("(t p) c -> p t c", p=P))
        emb_allT_sb = inp.tile([P, KT, N], MDT, tag="emb_allT")
        nc.sync.dma_start(
            out=emb_allT_sb, in_=emb_allT_d.rearrange("(t p) c -> p t c", p=P)
        )

        # ---- phase 0: PE warmup (un-throttle HAM before real work arrives) ----
        warm_sb = work.tile([P, KV], MDT, tag="warm")
        nc.vector.memset(warm_sb, 0.0)
        with tc.tile_pool(name="psW", bufs=1, space="PSUM") as psW:
            w_ps = psW.tile([P, KV], F32, tag="w")
            for _ in range(8):
                nc.tensor.matmul(
                    w_ps, warm_sb[:, 0:P], warm_sb, start=True, stop=True
                )

        # ---- phase 1: G = emb^T @ emb_all  -> [c(part,CT), kv] ----
        G_sb = work.tile([P, CT, KV], MDT, tag="G")
        with tc.tile_pool(name="psG", bufs=2, space="PSUM") as psG:
            for ct in range(CT):
                g_ps = psG.tile([P, KV], F32, tag="G")
                for nt in range(NT):
                    nc.tensor.matmul(
                        g_ps,
                        emb_sb[:, nt, ct * P : (ct + 1) * P],
                        emb_all_sb[:, nt, :],
                        start=(nt == 0),
                        stop=(nt == NT - 1),
                    )
                nc.vector.tensor_copy(G_sb[:, ct, :], g_ps)

        # ---- phase 2: per-head attention in channel space (sw-pipelined) ----
        Usum = work.tile([P, CT, KV], MDT, tag="Usum")
        A_t, SC_t, RS_t, ET_t, alpha_t, zr_t = {}, {}, {}, {}, {}, {}
        with (
            tc.tile_pool(name="psA", bufs=2, space="PSUM") as psA,
            tc.tile_pool(name="psS", bufs=2, space="PSUM") as psS,
            tc.tile_pool(name="psU", bufs=2, space="PSUM") as psU,
            tc.tile_pool(name="psSmall", bufs=2, space="PSUM") as psSmall,
        ):

            def emit_A(h):
                # A_hT[kv, d] = sum_c G[c, kv] * WqT[c, d]
                A_sb = hwork.tile([P, KT, C0], MDT, tag="A")
                A_t[h] = A_sb
                for mt in range(KT):
                    a_ps = psA.tile([P, C0], F32, tag="A")
                    for kc in range(CT):
                        nc.tensor.matmul(
                            a_ps,
                            G_sb[:, kc, mt * P : (mt + 1) * P],
                            wq_sb[h][:, kc, :],
                            start=(kc == 0),
                            stop=(kc == CT - 1),
                        )
                    nc.scalar.copy(A_sb[:, mt, :], a_ps)

            def emit_S(h):
                # S_hT[j, d] = sum_kv WkT[kv, j] * A_hT[kv, d]
                # variance stats via bn_stats per tile + bn_aggr (DVE only)
                SC = hwork.tile([P, KT, C0], MDT, tag="SC")
                BN = hwork.tile([P, KT, 6], F32, tag="BN")
                SC_t[h], RS_t[h] = SC, BN
                A_sb = A_t[h]
                for jm in range(KT):
                    s_ps = psS.tile([P, C0], F32, tag="S")
                    for kt in range(KT):
                        nc.tensor.matmul(
                            s_ps,
                            wk_sb[h][:, kt, jm * P : (jm + 1) * P],
                            A_sb[:, kt, :],
                            start=(kt == 0),
                            stop=(kt == KT - 1),
                        )
                    nc.vector.tensor_copy(SC[:, jm, :], s_ps)
                    nc.vector.bn_stats(out=BN[:, jm, :], in_=SC[:, jm, :])

            a2_t = {}

            def emit_stats1(h):
                # per-partition (mean, var) over this partition's 4x256 values,
                # then E2 = var + mean^2, grand-combine across partitions via a
                # tiny ones-matmul (all partitions hold equal counts -> exact).
                mv = small.tile([P, 2], F32, tag="mv")
                nc.vector.bn_aggr(out=mv, in_=RS_t[h])
                me2 = small.tile([P, 2], F32, tag="me2")
                nc.vector.tensor_copy(me2[:, 0:1], mv[:, 0:1])
                nc.vector.tensor_mul(me2[:, 1:2], mv[:, 0:1], mv[:, 0:1])
                nc.vector.tensor_add(me2[:, 1:2], me2[:, 1:2], mv[:, 1:2])
                cs_ps = psSmall.tile([P, 2], F32, tag="small")
                nc.tensor.matmul(cs_ps[0:2, :], ones2_f, me2, start=True, stop=True)
                m12 = small.tile([1, 2], F32, tag="m12")
                nc.vector.tensor_scalar_mul(m12, cs_ps[0:1, :], 1.0 / P)
                var = small.tile([1, 1], F32, tag="var")
                nc.vector.tensor_mul(var, m12[:, 0:1], m12[:, 0:1])
                nc.vector.tensor_sub(var, m12[:, 1:2], var)
                sd = small.tile([1, 1], F32, tag="sd")
                nc.scalar.activation(sd, var, Sqrt, bias=epsb[0:1, :])
                ar = small.tile([1, 1], F32, tag="ar")
                nc.vector.reciprocal(ar, sd)
                a2 = small.tile([1, 2], MDT, tag="a2")
                nc.vector.tensor_copy(a2[:, 0:1], ar)
                nc.vector.tensor_copy(a2[:, 1:2], ar)
                a2_t[h] = a2

            def emit_stats2(h):
                # broadcast alpha to all partitions, then ET = exp(alpha * S)
                ab_ps = psSmall.tile([P, 2], F32, tag="small")
                nc.tensor.matmul(ab_ps, ones_row, a2_t[h], start=True, stop=True)
                alpha = small.tile([P, 1], F32, tag="alpha")
                nc.vector.tensor_copy(alpha, ab_ps[:, 0:1])
                alpha_t[h] = alpha
                ET = hwork.tile([P, KT, C0], MDT, tag="ET")
                ET_t[h] = ET
                SC = SC_t[h]
                for jm in range(KT):
                    nc.scalar.activation(ET[:, jm, :], SC[:, jm, :], Exp, scale=alpha)

            def emit_ZU(h):
                ET = ET_t[h]
                # Z[d] = sum_j ET[j, d]; zr = 1/Z
                zr = small.tile([P, CT], F32, tag="zr")
                zr_t[h] = zr
                for dm in range(CT):
                    z_ps = psSmall.tile([P, 2], F32, tag="small")
                    for jm in range(KT):
                        nc.tensor.matmul(
                            z_ps,
                            ET[:, jm, dm * P : (dm + 1) * P],
                            ones2,
                            start=(jm == 0),
                            stop=(jm == KT - 1),
                        )
                    nc.vector.reciprocal(zr[:, dm : dm + 1], z_ps[:, 0:1])
                # U'[d, k] = sum_j ET[j, d] * Wv[j, k];  Usum += U' / Z
                for dm in range(CT):
                    u_ps = psU.tile([P, KV], F32, tag="U")
                    for jm in range(KT):
                        nc.tensor.matmul(
                            u_ps,
                            ET[:, jm, dm * P : (dm + 1) * P],
                            wv_sb[h][:, jm, :],
                            start=(jm == 0),
                            stop=(jm == KT - 1),
                        )
                    if h == 0:
                        nc.vector.tensor_scalar_mul(
                            Usum[:, dm, :], u_ps, zr[:, dm : dm + 1]
                        )
                    else:
                        ut = hwork.tile([P, KV], MDT, tag="ut")
                        nc.vector.tensor_scalar_mul(ut, u_ps, zr[:, dm : dm + 1])
                        nc.vector.tensor_add(Usum[:, dm, :], Usum[:, dm, :], ut)

            # skewed emission: PE stream never waits on a head's stats chain
            emit_A(0)
            emit_S(0)
            emit_A(1)
            emit_stats1(0)
            emit_S(1)
            emit_stats2(0)
            emit_A(2)
            emit_stats1(1)
            emit_ZU(0)
            emit_S(2)
            emit_stats2(1)
            emit_A(3)
            emit_stats1(2)
            emit_ZU(1)
            emit_S(3)
            emit_stats2(2)
            emit_ZU(2)
            emit_stats1(3)
            emit_stats2(3)
            emit_ZU(3)

        # ---- phase 3: MT[k, d'] = sum_d Usum[d, k] * WoutT[d, d'] ----
        MT_sb = work.tile([P, KT, C0], MDT, tag="MT")
        with tc.tile_pool(name="psMT", bufs=2, space="PSUM") as psMT:
            for km in range(KT):
                mt_ps = psMT.tile([P, C0], F32, tag="MT")
                for dt_ in range(CT):
                    nc.tensor.matmul(
                        mt_ps,
                        Usum[:, dt_, km * P : (km + 1) * P],
                        woutT_sb[:, dt_, :],
                        start=(dt_ == 0),
                        stop=(dt_ == CT - 1),
                    )
                nc.vector.tensor_copy(MT_sb[:, km, :], mt_ps)

        # ---- phase 4: o[n, d'] = sum_k emb_allT[k, n] * MT[k, d'] ----
        with (
            tc.tile_pool(name="psO", bufs=3, space="PSUM") as psO,
            tc.tile_pool(name="osb", bufs=3) as osb,
        ):
            o_r = o_d.rearrange("(t p) c -> p t c", p=P)
            for nm in range(NT):
                o_ps = psO.tile([P, C0], F32, tag="o")
                for kt in range(KT):
                    nc.tensor.matmul(
                        o_ps,
                        emb_allT_sb[:, kt, nm * P : (nm + 1) * P],
                        MT_sb[:, kt, :],
                        start=(kt == 0),
                        stop=(kt == KT - 1),
                    )
                ot = osb.tile([P, C0], F32, tag="o")
                if nm % 2 == 0:
                    nc.vector.tensor_copy(ot, o_ps)
                else:
                    nc.scalar.copy(ot, o_ps)
                nc.sync.dma_start(out=o_r[:, nm, :], in_=ot)

    nc.compile()
    return nc


_NC_CACHE: dict = {}


def _get_nc(mm_dtype: str = "float16"):
    if mm_dtype not in _NC_CACHE:
        _NC_CACHE[mm_dtype] = _build_nc(mm_dtype)
    return _NC_CACHE[mm_dtype]


def _make_in_maps(emb, emb_all, Wq, Wk, Wv, Wout, np_dt=np.float16):
    f = np.float32
    wqT = np.ascontiguousarray(np.asarray(Wq, f).transpose(0, 2, 1)).astype(np_dt)
    wkT = np.ascontiguousarray(np.asarray(Wk, f).transpose(0, 2, 1)).astype(np_dt)
    wv = np.ascontiguousarray(np.asarray(Wv, f)).astype(np_dt)
    woutT = np.ascontiguousarray(np.asarray(Wout, f).T * (1.0 / H)).astype(np_dt)
    in_maps = []
    for b in range(B):
        in_maps.append(
            dict(
                emb=np.asarray(emb[b], f).astype(np_dt),
                emb_all=np.asarray(emb_all[b], f).astype(np_dt),
                emb_allT=np.ascontiguousarray(np.asarray(emb_all[b], f).T).astype(
                    np_dt
                ),
                wqT=wqT,
                wkT=wkT,
                wv=wv,
                woutT=woutT,
            )
        )
    return in_maps


def run(inputs: dict, mm_dtype: str = "float16", **spmd_kwargs):
    """Run on the 8 NeuronCores; returns (output [B,N,C0], BassKernelResults)."""
    nc = _get_nc(mm_dtype)
    np_dt = mybir.dt.np(getattr(mybir.dt, mm_dtype))
    in_maps = _make_in_maps(**inputs, np_dt=np_dt)
    res = run_bass_kernel_spmd(nc, in_maps, list(range(NCORES)), **spmd_kwargs)
    out = np.stack([res.results[c]["o"] for c in range(NCORES)], axis=0)
    return out, res


def kernel(emb, emb_all, Wq, Wk, Wv, Wout):
    out, _ = run(dict(emb=emb, emb_all=emb_all, Wq=Wq, Wk=Wk, Wv=Wv, Wout=Wout))
    return out


# revision 12
# speedup vs baseline: 1.5524x; 1.0732x over previous
"""Trainium2 Bass kernel for nn_Attention1 (channel attention, B=8,N=1024,C0=256,KV=512,H=4).

Sharding: pure data-parallel over batch B=8 across the 8 NeuronCores (one batch
element per core, no collectives).

Algorithm (per core, batch element b), algebraically refactored so the N=1024
dimension is contracted once up front:

    G    = emb_b^T @ emb_all_b                      [C0, KV]   (gram matrix)
    per head h:
      A_hT = G^T @ Wq_h^T                           [KV, C0]
      S_hT = (Wk_h^T)-chain @ A_hT  (scores^T)      [KV, C0]
      alpha = 1/sqrt(var(S) + KV*eps)               (instance-norm; the mean
               cancels under softmax shift-invariance, so only var is needed;
               the 1/sqrt(KV) score scaling is folded into alpha)
      ET   = exp(alpha * S_hT)                      [KV, C0]
      Z_d  = sum_j ET[j, d]   (softmax denominators)
      U_h  = (ET^T @ Wv_h) / Z                      [C0, KV]
    Usum = sum_h U_h;  MT = Usum^T @ (Wout^T/H)     [KV, C0]
    o_b  = emb_all_b @ MT                           [N, C0]

This is exact (same math as the reference, ~3.6x fewer FLOPs) and needs zero
on-chip transposes: weights are pre-transposed on the host, and emb_all is
shipped both n-major (for G) and k-major (for the final projection).

The head loop is software-pipelined in emission order (A0 S0 A1 St0 S1 A2 ZU0
St1 S2 A3 ZU1 St2 S3 ZU2 St3 ZU3) so the TensorE instruction stream never
stalls on a head's cross-engine stats/softmax chain.  Map-wide variance stats
are fused into the PSUM->SBUF copy/square via ScalarE accum_out (per-partition
row sums) followed by one tiny ones-matmul per head.
"""

import sys

for _p in (
    "/root/.axon_site",
    "/root/.axon_site/_ro/trn_rl_repo",
    "/root/.axon_site/_ro/pypackages",
    "/opt/trn_rl_repo",
):
    if _p not in sys.path:
        sys.path.append(_p)

from contextlib import ExitStack

import numpy as np

import concourse.bass as bass
import concourse.tile as tile
from concourse import bacc, mybir
from concourse.bass_utils import run_bass_kernel_spmd

NCORES = 8
B, N, C0, KV, H = 8, 1024, 256, 512, 4
EPS = 1e-5
P = 128
NT, CT, KT = N // P, C0 // P, KV // P  # 8, 2, 4
CNT = C0 * KV  # elements per (b, h) score map

F32 = mybir.dt.float32


def _build_nc(mm_dtype: str = "float16"):
    """Build + compile the single-core program (same program on all 8 cores)."""
    nc = bacc.Bacc(
        "TRN2",
        target_bir_lowering=False,
        debug=False,
        num_devices=NCORES,
    )

    MDT = getattr(mybir.dt, mm_dtype)

    emb_d = nc.dram_tensor("emb", [N, C0], MDT, kind="ExternalInput").ap()
    emb_all_d = nc.dram_tensor("emb_all", [N, KV], MDT, kind="ExternalInput").ap()
    emb_allT_d = nc.dram_tensor("emb_allT", [KV, N], MDT, kind="ExternalInput").ap()
    wqT_d = nc.dram_tensor("wqT", [H, C0, C0], MDT, kind="ExternalInput").ap()
    wkT_d = nc.dram_tensor("wkT", [H, KV, KV], MDT, kind="ExternalInput").ap()
    wv_d = nc.dram_tensor("wv", [H, KV, KV], MDT, kind="ExternalInput").ap()
    woutT_d = nc.dram_tensor("woutT", [C0, C0], MDT, kind="ExternalInput").ap()
    o_d = nc.dram_tensor("o", [N, C0], F32, kind="ExternalOutput").ap()

    Exp = mybir.ActivationFunctionType.Exp
    Sqrt = mybir.ActivationFunctionType.Sqrt
    Square = mybir.ActivationFunctionType.Square
    Copy = mybir.ActivationFunctionType.Copy

    with tile.TileContext(nc) as tc, ExitStack() as ctx:
        const = ctx.enter_context(tc.tile_pool(name="const", bufs=1))
        inp = ctx.enter_context(tc.tile_pool(name="inp", bufs=1))
        work = ctx.enter_context(tc.tile_pool(name="work", bufs=1))
        hwork = ctx.enter_context(tc.tile_pool(name="hwork", bufs=3))
        small = ctx.enter_context(tc.tile_pool(name="small", bufs=2))

        ones2_f = const.tile([P, 2], F32, tag="ones2_f")
        nc.vector.memset(ones2_f, 1.0)
        ones2 = const.tile([P, 2], MDT, tag="ones2")
        nc.vector.tensor_copy(ones2, ones2_f)
        ones_row_f = const.tile([1, P], F32, tag="ones_row_f")
        nc.vector.memset(ones_row_f, 1.0)
        ones_row = const.tile([1, P], MDT, tag="ones_row")
        nc.vector.tensor_copy(ones_row, ones_row_f)
        epsb = const.tile([P, 1], F32, tag="epsb")
        nc.vector.memset(epsb, float(KV * EPS))

        # ---- resident input loads (order matters: feeds the pipeline) ----
        # per-tile chunks so the G matmuls can start as soon as chunk 0 lands
        emb_sb = inp.tile([P, NT, C0], MDT, tag="emb")
        emb_r = emb_d.rearrange("(t p) c -> p t c", p=P)
        emb_all_sb = inp.tile([P, NT, KV], MDT, tag="emb_all")
        emb_all_r = emb_all_d.rearrange("(t p) c -> p t c", p=P)
        for nt in range(NT):
            nc.sync.dma_start(out=emb_sb[:, nt, :], in_=emb_r[:, nt, :])
            nc.sync.dma_start(out=emb_all_sb[:, nt, :], in_=emb_all_r[:, nt, :])
        wq_sb, wk_sb, wv_sb = [], [], []
        emb_allT_sb = inp.tile([P, KT, N], MDT, tag="emb_allT")
        for h in range(H):
            t = inp.tile([P, CT, C0], MDT, tag=f"wq{h}")
            nc.sync.dma_start(out=t, in_=wqT_d[h].rearrange("(t p) c -> p t c", p=P))
            wq_sb.append(t)
            t = inp.tile([P, KT, KV], MDT, tag=f"wk{h}")
            nc.sync.dma_start(out=t, in_=wkT_d[h].rearrange("(t p) c -> p t c", p=P))
            wk_sb.append(t)
            t = inp.tile([P, KT, KV], MDT, tag=f"wv{h}")
            nc.sync.dma_start(out=t, in_=wv_d[h].rearrange("(t p) c -> p t c", p=P))
            wv_sb.append(t)
            if h == 1:
                nc.sync.dma_start(
                    out=emb_allT_sb,
                    in_=emb_allT_d.rearrange("(t p) c -> p t c", p=P),
                )
        woutT_sb = inp.tile([P, CT, C0], MDT, tag="woutT")
        nc.sync.dma_start(out=woutT_sb, in_=woutT_d.rearrange("(t p) c -> p t c", p=P))

        # ---- phase 0: PE warmup (un-throttle HAM before real work arrives) ----
        warm_sb = work.tile([P, KV], MDT, tag="warm")
        nc.vector.memset(warm_sb, 0.0)
        with tc.tile_pool(name="psW", bufs=1, space="PSUM") as psW:
            w_ps = psW.tile([P, KV], F32, tag="w")
            for _ in range(6):
                nc.tensor.matmul(
                    w_ps, warm_sb[:, 0:P], warm_sb, start=True, stop=True
                )

        # ---- phase 1: G = emb^T @ emb_all  -> [c(part,CT), kv] ----
        G_sb = work.tile([P, CT, KV], MDT, tag="G")
        with tc.tile_pool(name="psG", bufs=2, space="PSUM") as psG:
            for ct in range(CT):
                g_ps = psG.tile([P, KV], F32, tag="G")
                for nt in range(NT):
                    nc.tensor.matmul(
                        g_ps,
                        emb_sb[:, nt, ct * P : (ct + 1) * P],
                        emb_all_sb[:, nt, :],
                        start=(nt == 0),
                        stop=(nt == NT - 1),
                    )
                nc.vector.tensor_copy(G_sb[:, ct, :], g_ps)

        # ---- phase 2: per-head attention in channel space (sw-pipelined) ----
        Usum = work.tile([P, CT, KV], MDT, tag="Usum")
        A_t, SC_t, RS_t, ET_t, alpha_t, zr_t = {}, {}, {}, {}, {}, {}
        with (
            tc.tile_pool(name="psA", bufs=2, space="PSUM") as psA,
            tc.tile_pool(name="psS", bufs=2, space="PSUM") as psS,
            tc.tile_pool(name="psU", bufs=2, space="PSUM") as psU,
            tc.tile_pool(name="psSmall", bufs=2, space="PSUM") as psSmall,
        ):

            def emit_A(h):
                # A_hT[kv, d] = sum_c G[c, kv] * WqT[c, d]
                A_sb = hwork.tile([P, KT, C0], MDT, tag="A")
                A_t[h] = A_sb
                for mt in range(KT):
                    a_ps = psA.tile([P, C0], F32, tag="A")
                    for kc in range(CT):
                        nc.tensor.matmul(
                            a_ps,
                            G_sb[:, kc, mt * P : (mt + 1) * P],
                            wq_sb[h][:, kc, :],
                            start=(kc == 0),
                            stop=(kc == CT - 1),
                        )
                    nc.scalar.copy(A_sb[:, mt, :], a_ps)

            def emit_S(h):
                # S_hT[j, d] = sum_kv WkT[kv, j] * A_hT[kv, d]
                # variance stats via bn_stats per tile + bn_aggr (DVE only)
                SC = hwork.tile([P, KT, C0], MDT, tag="SC")
                BN = hwork.tile([P, KT, 6], F32, tag="BN")
                SC_t[h], RS_t[h] = SC, BN
                A_sb = A_t[h]
                for jm in range(KT):
                    s_ps = psS.tile([P, C0], F32, tag="S")
                    for kt in range(KT):
                        nc.tensor.matmul(
                            s_ps,
                            wk_sb[h][:, kt, jm * P : (jm + 1) * P],
                            A_sb[:, kt, :],
                            start=(kt == 0),
                            stop=(kt == KT - 1),
                        )
                    nc.vector.tensor_copy(SC[:, jm, :], s_ps)
                    nc.vector.bn_stats(out=BN[:, jm, :], in_=SC[:, jm, :])

            a2_t = {}

            def emit_stats1(h):
                # per-partition (mean, var) over this partition's 4x256 values,
                # then E2 = var + mean^2, grand-combine across partitions via a
                # tiny ones-matmul (all partitions hold equal counts -> exact).
                mv = small.tile([P, 2], F32, tag="mv")
                nc.vector.bn_aggr(out=mv, in_=RS_t[h])
                me2 = small.tile([P, 2], F32, tag="me2")
                nc.vector.tensor_copy(me2[:, 0:1], mv[:, 0:1])
                nc.vector.tensor_mul(me2[:, 1:2], mv[:, 0:1], mv[:, 0:1])
                nc.vector.tensor_add(me2[:, 1:2], me2[:, 1:2], mv[:, 1:2])
                cs_ps = psSmall.tile([P, 2], F32, tag="small")
                nc.tensor.matmul(cs_ps[0:2, :], ones2_f, me2, start=True, stop=True)
                m12 = small.tile([1, 2], F32, tag="m12")
                nc.vector.tensor_scalar_mul(m12, cs_ps[0:1, :], 1.0 / P)
                var = small.tile([1, 1], F32, tag="var")
                nc.vector.tensor_mul(var, m12[:, 0:1], m12[:, 0:1])
                nc.vector.tensor_sub(var, m12[:, 1:2], var)
                sd = small.tile([1, 1], F32, tag="sd")
                nc.scalar.activation(sd, var, Sqrt, bias=epsb[0:1, :])
                ar = small.tile([1, 1], F32, tag="ar")
                nc.vector.reciprocal(ar, sd)
                a2 = small.tile([1, 2], MDT, tag="a2")
                nc.vector.tensor_copy(a2, ar.to_broadcast((1, 2)))
                a2_t[h] = a2

            def emit_stats2(h):
                # broadcast alpha to all partitions, then ET = exp(alpha * S)
                ab_ps = psSmall.tile([P, 2], F32, tag="small")
                nc.tensor.matmul(ab_ps, ones_row, a2_t[h], start=True, stop=True)
                alpha = small.tile([P, 1], F32, tag="alpha")
                nc.vector.tensor_copy(alpha, ab_ps[:, 0:1])
                alpha_t[h] = alpha
                ET = hwork.tile([P, KT, C0], MDT, tag="ET")
                ET_t[h] = ET
                SC = SC_t[h]
                for jm in range(KT):
                    nc.scalar.activation(ET[:, jm, :], SC[:, jm, :], Exp, scale=alpha)

            def emit_ZU(h):
                ET = ET_t[h]
                # Z[d] = sum_j ET[j, d]; zr = 1/Z
                zr = small.tile([P, CT], F32, tag="zr")
                zr_t[h] = zr
                for dm in range(CT):
                    z_ps = psSmall.tile([P, 2], F32, tag="small")
                    for jm in range(KT):
                        nc.tensor.matmul(
                            z_ps,
                            ET[:, jm, dm * P : (dm + 1) * P],
                            ones2,
                            start=(jm == 0),
                            stop=(jm == KT - 1),
                        )
                    nc.vector.reciprocal(zr[:, dm : dm + 1], z_ps[:, 0:1])
                # U'[d, k] = sum_j ET[j, d] * Wv[j, k];  Usum += U' / Z
                for dm in range(CT):
                    u_ps = psU.tile([P, KV], F32, tag="U")
                    for jm in range(KT):
                        nc.tensor.matmul(
                            u_ps,
                            ET[:, jm, dm * P : (dm + 1) * P],
                            wv_sb[h][:, jm, :],
                            start=(jm == 0),
                            stop=(jm == KT - 1),
                        )
                    if h == 0:
                        nc.vector.tensor_scalar_mul(
                            Usum[:, dm, :], u_ps, zr[:, dm : dm + 1]
                        )
                    else:
                        ut = hwork.tile([P, KV], MDT, tag="ut")
                        nc.vector.tensor_scalar_mul(ut, u_ps, zr[:, dm : dm + 1])
                        nc.vector.tensor_add(Usum[:, dm, :], Usum[:, dm, :], ut)

            # skewed emission: PE stream never waits on a head's stats chain
            emit_A(0)
            emit_S(0)
            emit_A(1)
            emit_S(1)
            emit_stats1(0)
            emit_A(2)
            emit_S(2)
            emit_stats2(0)
            emit_stats1(1)
            emit_A(3)
            emit_ZU(0)
            emit_S(3)
            emit_stats2(1)
            emit_stats1(2)
            emit_ZU(1)
            emit_stats2(2)
            emit_stats1(3)
            emit_ZU(2)
            emit_stats2(3)
            emit_ZU(3)

        # ---- phase 3: MT[k, d'] = sum_d Usum[d, k] * WoutT[d, d'] ----
        MT_sb = work.tile([P, KT, C0], MDT, tag="MT")
        with tc.tile_pool(name="psMT", bufs=2, space="PSUM") as psMT:
            for km in range(KT):
                mt_ps = psMT.tile([P, C0], F32, tag="MT")
                for dt_ in range(CT):
                    nc.tensor.matmul(
                        mt_ps,
                        Usum[:, dt_, km * P : (km + 1) * P],
                        woutT_sb[:, dt_, :],
                        start=(dt_ == 0),
                        stop=(dt_ == CT - 1),
                    )
                nc.vector.tensor_copy(MT_sb[:, km, :], mt_ps)

        # ---- phase 4: o[n, d'] = sum_k emb_allT[k, n] * MT[k, d'] ----
        with (
            tc.tile_pool(name="psO", bufs=3, space="PSUM") as psO,
            tc.tile_pool(name="osb", bufs=3) as osb,
        ):
            o_r = o_d.rearrange("(t p) c -> p t c", p=P)
            for nm in range(NT):
                o_ps = psO.tile([P, C0], F32, tag="o")
                for kt in range(KT):
                    nc.tensor.matmul(
                        o_ps,
                        emb_allT_sb[:, kt, nm * P : (nm + 1) * P],
                        MT_sb[:, kt, :],
                        start=(kt == 0),
                        stop=(kt == KT - 1),
                    )
                ot = osb.tile([P, C0], F32, tag="o")
                if nm % 2 == 0:
                    nc.vector.tensor_copy(ot, o_ps)
                else:
                    nc.scalar.copy(ot, o_ps)
                nc.sync.dma_start(out=o_r[:, nm, :], in_=ot)

    nc.compile()
    return nc


_NC_CACHE: dict = {}


def _get_nc(mm_dtype: str = "float16"):
    if mm_dtype not in _NC_CACHE:
        _NC_CACHE[mm_dtype] = _build_nc(mm_dtype)
    return _NC_CACHE[mm_dtype]


def _make_in_maps(emb, emb_all, Wq, Wk, Wv, Wout, np_dt=np.float16):
    f = np.float32
    wqT = np.ascontiguousarray(np.asarray(Wq, f).transpose(0, 2, 1)).astype(np_dt)
    wkT = np.ascontiguousarray(np.asarray(Wk, f).transpose(0, 2, 1)).astype(np_dt)
    wv = np.ascontiguousarray(np.asarray(Wv, f)).astype(np_dt)
    woutT = np.ascontiguousarray(np.asarray(Wout, f).T * (1.0 / H)).astype(np_dt)
    in_maps = []
    for b in range(B):
        in_maps.append(
            dict(
                emb=np.asarray(emb[b], f).astype(np_dt),
                emb_all=np.asarray(emb_all[b], f).astype(np_dt),
                emb_allT=np.ascontiguousarray(np.asarray(emb_all[b], f).T).astype(
                    np_dt
                ),
                wqT=wqT,
                wkT=wkT,
                wv=wv,
                woutT=woutT,
            )
        )
    return in_maps


def run(inputs: dict, mm_dtype: str = "float16", **spmd_kwargs):
    """Run on the 8 NeuronCores; returns (output [B,N,C0], BassKernelResults)."""
    nc = _get_nc(mm_dtype)
    np_dt = mybir.dt.np(getattr(mybir.dt, mm_dtype))
    in_maps = _make_in_maps(**inputs, np_dt=np_dt)
    res = run_bass_kernel_spmd(nc, in_maps, list(range(NCORES)), **spmd_kwargs)
    out = np.stack([res.results[c]["o"] for c in range(NCORES)], axis=0)
    return out, res


def kernel(emb, emb_all, Wq, Wk, Wv, Wout):
    out, _ = run(dict(emb=emb, emb_all=emb_all, Wq=Wq, Wk=Wk, Wv=Wv, Wout=Wout))
    return out
